# revision 23
# baseline (speedup 1.0000x reference)
"""Trainium2 Bass kernel for a single-query masked-attention module.

Computes, for batch b:
    x        = hidden @ W1.T                                   [bsz, out]
    score    = einsum('bsd,bd->sb', encoder_outs, x)           [seq, bsz]
    masked   = mask * score ; exact zeros -> -1e10
    attn     = softmax(masked, axis=seq)                       [seq, bsz]
    ctx      = einsum('sb,bsd->bd', attn, encoder_outs)        [bsz, out]
    out      = tanh(concat(ctx, hidden) @ W2.T)                [bsz, out]
returns (out, attn).

Sharding: data-parallel over batch across 8 NeuronCores (8 batches/core).
Batches are sorted by src_len and dealt so that every core's pipeline
slot g holds batches of similar length; per-slot effective sequence
lengths (group max, rounded up to 128) are compiled into the module.
Everything past the effective length is exactly zero after softmax in
fp32 (exp(-1e10 - max) underflows to 0), so skipping it is exact.
"""

import numpy as np
import ml_dtypes

P = 128
NCORES = 8
BPC = 8  # batches per core
BSZ, SEQ, HID, OUT = 64, 1024, 1024, 1024
CAT = HID + OUT
DT = OUT // P   # d tiles per batch
HT = HID // P   # h tiles
CT = CAT // P   # concat tiles
NEG = -1e10

_module_cache: dict = {}


def _build_module(seq_effs):
    """Build the per-core Bass module. seq_effs: tuple of BPC effective
    sequence lengths (multiples of 128), one per pipeline slot."""
    import concourse.bass as bass
    import concourse.mybir as mybir
    import concourse.tile as tile
    f32 = mybir.dt.float32
    bf16 = mybir.dt.float16  # fp16: same PE rate as bf16, 8x finer mantissa
    bfl = mybir.dt.bfloat16
    Alu = mybir.AluOpType
    Act = mybir.ActivationFunctionType

    nc = bass.Bass()
    enct_d = nc.dram_tensor("enct", [BPC, OUT, SEQ], f32, kind="ExternalInput")
    hidt_d = nc.dram_tensor("hidt", [HID, BPC], f32, kind="ExternalInput")
    hidtb_d = nc.dram_tensor("hidtb", [HID, BPC], bf16, kind="ExternalInput")
    w1t_d = nc.dram_tensor("w1t", [HID, OUT], f32, kind="ExternalInput")
    w2tb_d = nc.dram_tensor("w2tb", [CAT, OUT], bf16, kind="ExternalInput")
    amask_d = nc.dram_tensor("amask", [BPC, SEQ], bfl, kind="ExternalInput")
    out2_d = nc.dram_tensor("out2", [BPC, OUT], f32, kind="ExternalOutput")
    attn_d = nc.dram_tensor("attn", [BPC, SEQ], f32, kind="ExternalOutput")

    with tile.TileContext(nc) as tc:
        with (
            tc.tile_pool(name="big", bufs=24) as big,
            tc.tile_pool(name="w2p", bufs=1) as w2p,
            tc.tile_pool(name="small", bufs=1) as small,
            tc.tile_pool(name="soft", bufs=2) as soft,
            tc.tile_pool(name="bcp", bufs=2) as bcp,
            tc.tile_pool(name="dramp", bufs=2, space="DRAM") as dramp,
            tc.tile_pool(name="pss", bufs=2, space="PSUM") as pss,
            tc.tile_pool(name="psx", bufs=1, space="PSUM") as psx,
        ):
            # ---- weights and per-core constants
            w1_tiles = []
            for j in range(HT):
                t = big.tile([P, OUT], f32, tag="bigtile")
                nc.sync.dma_start(out=t[:], in_=w1t_d[j * P:(j + 1) * P, :])
                w1_tiles.append(t)
            hidt_tiles = []
            for j in range(HT):
                t = small.tile([P, BPC], f32, tag=f"hidt{j}")
                nc.sync.dma_start(out=t[:], in_=hidt_d[j * P:(j + 1) * P, :])
                hidt_tiles.append(t)
            w2_tiles = []
            for k in range(CT):
                t = w2p.tile([P, OUT], bf16, tag=f"w2_{k}")
                nc.sync.dma_start(out=t[:], in_=w2tb_d[k * P:(k + 1) * P, :])
                w2_tiles.append(t)
            # concat(ctx, hidden).T as bf16 column tiles; lower half is hidden
            catb_tiles = [
                small.tile([P, BPC], bf16, tag=f"catb{k}", name=f"catb{k}")
                for k in range(CT)
            ]
            for k in range(DT, CT):
                nc.sync.dma_start(
                    out=catb_tiles[k][:],
                    in_=hidtb_d[(k - DT) * P:(k - DT + 1) * P, :],
                )
            ctxf_tiles = [
                small.tile([P, BPC], f32, tag=f"ctxf{k}", name=f"ctxf{k}")
                for k in range(DT)
            ]
            ones1 = small.tile([1, 1], bfl, tag="ones1", name="ones1")
            nc.vector.memset(ones1[0:1, 0:1], 1.0)

            # ---- x.T = W1 @ hidden.T  -> DT tiles of [128(d), BPC]
            xt_tiles = []
            for k in range(DT):
                ps = psx.tile([P, BPC], f32, tag="psxT", bufs=2)
                for j in range(HT):
                    nc.tensor.matmul(
                        ps[:],
                        w1_tiles[j][:, k * P:(k + 1) * P],
                        hidt_tiles[j][:],
                        start=(j == 0),
                        stop=(j == HT - 1),
                    )
                xt = small.tile([P, BPC], f32, tag=f"xt{k}")
                nc.vector.tensor_copy(out=xt[:], in_=ps[:])
                xt_tiles.append(xt)

            # ---- per-batch pipeline
            for b in range(BPC):
                se = int(seq_effs[b])
                enc_tiles = []
                for k in range(DT):
                    t = big.tile([P, SEQ], f32, tag="bigtile")
                    nc.sync.dma_start(
                        out=t[:, 0:se], in_=enct_d[b, k * P:(k + 1) * P, 0:se]
                    )
                    enc_tiles.append(t)

                # scores[s] = sum_d x[b,d] * encT[d,s] + amask[s]
                # (the mask-add rides the PE accumulation as a K=1 term)
                amask = soft.tile([1, SEQ], bfl, tag="amask")
                nc.sync.dma_start(out=amask[0:1, 0:se], in_=amask_d[b:b + 1, 0:se])
                ps_s = pss.tile([1, SEQ], f32, tag="scores")
                for k in range(DT):
                    n0 = 0
                    while n0 < se:
                        n1 = min(se, n0 + 512)
                        nc.tensor.matmul(
                            ps_s[0:1, n0:n1],
                            xt_tiles[k][:, b:b + 1],
                            enc_tiles[k][:, n0:n1],
                            start=(k == 0),
                            stop=False,
                        )
                        n0 = n1
                n0 = 0
                while n0 < se:
                    n1 = min(se, n0 + 512)
                    nc.tensor.matmul(
                        ps_s[0:1, n0:n1],
                        ones1[0:1, 0:1],
                        amask[0:1, n0:n1],
                        start=False,
                        stop=True,
                    )
                    n0 = n1

                stats = soft.tile([1, 8], f32, tag="stats")
                nc.vector.tensor_reduce(
                    out=stats[0:1, 0:1],
                    in_=ps_s[0:1, 0:se],
                    axis=mybir.AxisListType.X,
                    op=Alu.max,
                )
                nc.vector.tensor_scalar_mul(
                    out=stats[0:1, 1:2], in0=stats[0:1, 0:1], scalar1=-1.0
                )
                # exp(masked - max) and its sum, fused on the scalar engine
                expv = soft.tile([1, SEQ], f32, tag="expv")
                nc.scalar.activation(
                    out=expv[0:1, 0:se],
                    in_=ps_s[0:1, 0:se],
                    func=Act.Exp,
                    bias=stats[0:1, 1:2],
                    scale=1.0,
                    accum_out=stats[0:1, 2:3],
                )
                nc.vector.reciprocal(out=stats[0:1, 3:4], in_=stats[0:1, 2:3])
                attn_n = soft.tile([1, SEQ], f32, tag="attn_n")
                nc.vector.tensor_scalar_mul(
                    out=attn_n[0:1, 0:se], in0=expv[0:1, 0:se], scalar1=stats[0:1, 3:4]
                )
                nc.sync.dma_start(out=attn_d[b:b + 1, 0:se], in_=attn_n[0:1, 0:se])

                # broadcast attn row across all 128 partitions: SBUF row ->
                # DRAM scratch -> stride-0 partition-replicated load
                adram = dramp.tile([1, SEQ], f32, tag="adram")
                nc.sync.dma_start(out=adram[0:1, 0:se], in_=attn_n[0:1, 0:se])
                abc = bcp.tile([P, SEQ], f32, tag="abc")
                src = adram[0:1, 0:se]
                bsrc = bass.AP(
                    tensor=src.tensor, offset=src.offset, ap=[[0, P], src.ap[-1]]
                )
                nc.sync.dma_start(out=abc[:, 0:se], in_=bsrc)

                # ctx.T[d, b] = sum_s encT[d, s] * attn[s]  (fused mul+reduce)
                for k in range(DT):
                    scr = bcp.tile([P, SEQ], f32, tag="scr")
                    nc.vector.scalar_tensor_tensor(
                        out=scr[:, 0:se],
                        in0=enc_tiles[k][:, 0:se],
                        scalar=1.0,
                        in1=abc[:, 0:se],
                        op0=Alu.mult,
                        op1=Alu.mult,
                        accum_out=ctxf_tiles[k][:, b:b + 1],
                    )

            # ---- out = tanh(cat.T.T @ W2.T) ; cat upper=ctx, lower=hidden
            for k in range(DT):
                nc.vector.tensor_copy(out=catb_tiles[k][:], in_=ctxf_tiles[k][:])
            ps_o = psx.tile([BPC, OUT], f32, tag="psout")
            for k in range(CT):
                for n in range(OUT // 512):
                    nc.tensor.matmul(
                        ps_o[0:BPC, n * 512:(n + 1) * 512],
                        catb_tiles[k][:],
                        w2_tiles[k][:, n * 512:(n + 1) * 512],
                        start=(k == 0),
                        stop=(k == CT - 1),
                    )
            out2_sb = small.tile([BPC, OUT], f32, tag="out2")
            nc.scalar.activation(
                out=out2_sb[:], in_=ps_o[:], func=Act.Tanh, scale=1.0
            )
            nc.sync.dma_start(out=out2_d[:, :], in_=out2_sb[:])

    # TRN2 instructions can carry at most one sync wait (walrus rejects
    # more); split multi-wait instructions into event-semaphore pairs.
    import bass_rust as _bass_rust
    _bass_rust.generate_event_semaphores(nc)
    return nc


def _get_module(seq_effs):
    key = tuple(int(x) for x in seq_effs)
    if key not in _module_cache:
        _module_cache[key] = _build_module(key)
    return _module_cache[key]


def _plan(lens):
    """Sort batches by length desc, deal one per core per slot.
    Returns (assign[core][slot] -> batch index, seq_eff[slot])."""
    order = np.argsort(-lens, kind="stable")
    assign = [[0] * BPC for _ in range(NCORES)]
    seq_effs = []
    for g in range(BPC):
        grp = order[g * NCORES:(g + 1) * NCORES]
        gmax = int(lens[grp].max())
        seq_effs.append(min(SEQ, max(P, ((gmax + P - 1) // P) * P)))
        for c in range(NCORES):
            assign[c][g] = int(grp[c])
    return assign, tuple(seq_effs)


def prepare(hidden, encoder_outs, src_lens, W1, W2):
    """Host-side sharding/layout prep. Returns (nc, in_maps, assign)."""
    hidden = np.asarray(hidden, dtype=np.float32)
    encoder_outs = np.asarray(encoder_outs, dtype=np.float32)
    lens = np.asarray(src_lens).astype(np.int64).clip(1, SEQ)
    W1 = np.asarray(W1, dtype=np.float32)
    W2 = np.asarray(W2, dtype=np.float32)

    assign, seq_effs = _plan(lens)
    nc = _get_module(seq_effs)

    w1t = np.ascontiguousarray(W1.T)                      # [h, o]
    w2tb = np.ascontiguousarray(W2.T).astype(np.float16)  # [c, o]
    iota = np.arange(SEQ)
    addmask_full = np.where(
        iota[None, :] < lens[:, None], 0.0, NEG
    ).astype(np.float32)

    in_maps = []
    for c in range(NCORES):
        idx = assign[c]
        enc_c = encoder_outs[idx]                         # [BPC, s, d]
        enct = np.ascontiguousarray(enc_c.transpose(0, 2, 1))  # [BPC, d, s]
        hidt = np.ascontiguousarray(hidden[idx].T)        # [h, BPC]
        in_maps.append({
            "enct": enct,
            "hidt": hidt,
            "hidtb": hidt.astype(np.float16),
            "w1t": w1t,
            "w2tb": w2tb,
            "amask": np.ascontiguousarray(addmask_full[idx]).astype(
                ml_dtypes.bfloat16
            ),
        })
    return nc, in_maps, assign


def kernel(hidden, encoder_outs, src_lens, W1, W2, _trace=False):
    from concourse.bass_utils import run_bass_kernel_spmd

    nc, in_maps, assign = prepare(hidden, encoder_outs, src_lens, W1, W2)
    res = run_bass_kernel_spmd(
        nc, in_maps, core_ids=list(range(NCORES)), trace=_trace
    )

    out = np.zeros((BSZ, OUT), dtype=np.float32)
    attn = np.zeros((BSZ, SEQ), dtype=np.float32)
    for c in range(NCORES):
        r = res.results[c]
        for g in range(BPC):
            bi = assign[c][g]
            out[bi] = r["out2"][g]
            attn[bi] = r["attn"][g]

    attn_scores = np.ascontiguousarray(attn.T)            # [seq, bsz]
    if _trace:
        return (out, attn_scores), res
    return out, attn_scores


# revision 31
# speedup vs baseline: 1.0986x; 1.0986x over previous
"""Trainium2 Bass kernel for a single-query masked-attention module.

Computes, for batch b:
    x        = hidden @ W1.T                                   [bsz, out]
    score    = einsum('bsd,bd->sb', encoder_outs, x)           [seq, bsz]
    masked   = mask * score ; exact zeros -> -1e10
    attn     = softmax(masked, axis=seq)                       [seq, bsz]
    ctx      = einsum('sb,bsd->bd', attn, encoder_outs)        [bsz, out]
    out      = tanh(concat(ctx, hidden) @ W2.T)                [bsz, out]
returns (out, attn).

Sharding: data-parallel over batch across 8 NeuronCores (8 batches/core).
Batches are sorted by src_len and dealt so that every core's pipeline
slot g holds batches of similar length; per-slot effective sequence
lengths (group max, rounded up to 128) are compiled into the module.
Everything past the effective length is exactly zero after softmax in
fp32 (exp(-1e10 - max) underflows to 0), so skipping it is exact.
"""

import numpy as np
import ml_dtypes

P = 128
NCORES = 8
BPC = 8  # batches per core
BSZ, SEQ, HID, OUT = 64, 1024, 1024, 1024
CAT = HID + OUT
DT = OUT // P   # d tiles per batch
HT = HID // P   # h tiles
CT = CAT // P   # concat tiles
NEG = -1e10

_module_cache: dict = {}


def _build_module(seq_effs):
    """Build the per-core Bass module. seq_effs: tuple of BPC effective
    sequence lengths (multiples of 128), one per pipeline slot."""
    import concourse.bass as bass
    import concourse.mybir as mybir
    import concourse.tile as tile
    from concourse.masks import make_identity
    f32 = mybir.dt.float32
    bf16 = mybir.dt.float16  # fp16: same PE rate as bf16, 8x finer mantissa
    bfl = mybir.dt.bfloat16
    Alu = mybir.AluOpType
    Act = mybir.ActivationFunctionType

    nc = bass.Bass()
    enct_d = nc.dram_tensor("enct", [BPC, OUT, SEQ], f32, kind="ExternalInput")
    hidt_d = nc.dram_tensor("hidt", [HID, BPC], f32, kind="ExternalInput")
    hidtb_d = nc.dram_tensor("hidtb", [HID, BPC], bf16, kind="ExternalInput")
    w1t_d = nc.dram_tensor("w1t", [HID, OUT], f32, kind="ExternalInput")
    w2tb_d = nc.dram_tensor("w2tb", [CAT, OUT], bf16, kind="ExternalInput")
    amask_d = nc.dram_tensor("amask", [BPC, SEQ], bfl, kind="ExternalInput")
    out2_d = nc.dram_tensor("out2", [BPC, OUT], f32, kind="ExternalOutput")
    attn_d = nc.dram_tensor("attn", [BPC, SEQ], f32, kind="ExternalOutput")

    with tile.TileContext(nc) as tc:
        with (
            tc.tile_pool(name="big", bufs=24) as big,
            tc.tile_pool(name="w2p", bufs=1) as w2p,
            tc.tile_pool(name="small", bufs=1) as small,
            tc.tile_pool(name="soft", bufs=2) as soft,
            tc.tile_pool(name="bcp", bufs=2) as bcp,
            tc.tile_pool(name="dramp", bufs=2, space="DRAM") as dramp,
            tc.tile_pool(name="pss", bufs=3, space="PSUM") as pss,
            tc.tile_pool(name="psx", bufs=1, space="PSUM") as psx,
        ):
            # ---- weights and per-core constants
            w1_tiles = []
            for j in range(HT):
                t = big.tile([P, OUT], f32, tag="bigtile")
                nc.sync.dma_start(out=t[:], in_=w1t_d[j * P:(j + 1) * P, :])
                w1_tiles.append(t)
            hidt_tiles = []
            for j in range(HT):
                t = small.tile([P, BPC], f32, tag=f"hidt{j}")
                nc.sync.dma_start(out=t[:], in_=hidt_d[j * P:(j + 1) * P, :])
                hidt_tiles.append(t)
            # concat(ctx, hidden).T as fp16 column tiles; lower half is hidden
            catb_tiles = [
                small.tile([P, BPC], bf16, tag=f"catb{k}", name=f"catb{k}")
                for k in range(CT)
            ]
            ctxf_tiles = [
                small.tile([P, BPC], f32, tag=f"ctxf{k}", name=f"ctxf{k}")
                for k in range(DT)
            ]
            ones1 = small.tile([1, 1], bfl, tag="ones1", name="ones1")
            nc.vector.memset(ones1[0:1, 0:1], 1.0)

            # ---- x = hidden @ W1.T with the tiny hidT as stationary weights
            # (weight-load-bound the other way round), then PE-transpose to
            # x.T column tiles for the score matmuls.
            xps = pss.tile([BPC, OUT], f32, tag="scores", name="xps")
            for j in range(HT):
                for n in range(OUT // 512):
                    nc.tensor.matmul(
                        xps[0:BPC, n * 512:(n + 1) * 512],
                        hidt_tiles[j][:],
                        w1_tiles[j][:, n * 512:(n + 1) * 512],
                        start=(j == 0),
                        stop=(j == HT - 1),
                    )
            x_sb = small.tile([BPC, OUT], f32, tag="x_sb")
            nc.scalar.copy(out=x_sb[:], in_=xps[:])
            ident = small.tile([P, P], f32, tag="ident")
            make_identity(nc, ident[:])
            xt_tiles = []
            for k in range(DT):
                tp = psx.tile([P, BPC], f32, tag="xtp", bufs=2)
                nc.tensor.transpose(
                    tp[:], x_sb[0:BPC, k * P:(k + 1) * P], ident[0:BPC, 0:BPC]
                )
                xt = small.tile([P, BPC], f32, tag=f"xt{k}")
                nc.vector.tensor_copy(out=xt[:], in_=tp[:])
                xt_tiles.append(xt)

            # ---- per-batch pipeline
            w2_tiles = []
            for b in range(BPC):
                if b == 1:
                    # W2 / hidden-cast loads are only needed at the end;
                    # emit them after batch 0 so startup DMA bandwidth goes
                    # to W1 + the first enc tiles.
                    for k in range(CT):
                        t = w2p.tile([P, OUT], bf16, tag=f"w2_{k}", name=f"w2_{k}")
                        nc.sync.dma_start(
                            out=t[:], in_=w2tb_d[k * P:(k + 1) * P, :]
                        )
                        w2_tiles.append(t)
                    for k in range(DT, CT):
                        nc.sync.dma_start(
                            out=catb_tiles[k][:],
                            in_=hidtb_d[(k - DT) * P:(k - DT + 1) * P, :],
                        )
                se = int(seq_effs[b])
                enc_tiles = []
                for k in range(DT):
                    t = big.tile([P, SEQ], f32, tag="bigtile")
                    nc.sync.dma_start(
                        out=t[:, 0:se], in_=enct_d[b, k * P:(k + 1) * P, 0:se]
                    )
                    enc_tiles.append(t)

                # scores[s] = sum_d x[b,d] * encT[d,s] + amask[s]
                # (the mask-add rides the PE accumulation as a K=1 term)
                amask = soft.tile([1, SEQ], bfl, tag="amask")
                nc.sync.dma_start(out=amask[0:1, 0:se], in_=amask_d[b:b + 1, 0:se])
                ps_s = pss.tile([1, SEQ], f32, tag="scores")
                for k in range(DT):
                    n0 = 0
                    while n0 < se:
                        n1 = min(se, n0 + 512)
                        nc.tensor.matmul(
                            ps_s[0:1, n0:n1],
                            xt_tiles[k][:, b:b + 1],
                            enc_tiles[k][:, n0:n1],
                            start=(k == 0),
                            stop=False,
                        )
                        n0 = n1
                n0 = 0
                while n0 < se:
                    n1 = min(se, n0 + 512)
                    nc.tensor.matmul(
                        ps_s[0:1, n0:n1],
                        ones1[0:1, 0:1],
                        amask[0:1, n0:n1],
                        start=False,
                        stop=True,
                    )
                    n0 = n1

                stats = soft.tile([1, 8], f32, tag="stats")
                nc.vector.tensor_reduce(
                    out=stats[0:1, 0:1],
                    in_=ps_s[0:1, 0:se],
                    axis=mybir.AxisListType.X,
                    op=Alu.max,
                )
                nc.vector.tensor_scalar_mul(
                    out=stats[0:1, 1:2], in0=stats[0:1, 0:1], scalar1=-1.0
                )
                # exp(masked - max) and its sum, fused on the scalar engine
                expv = soft.tile([1, SEQ], f32, tag="expv")
                nc.scalar.activation(
                    out=expv[0:1, 0:se],
                    in_=ps_s[0:1, 0:se],
                    func=Act.Exp,
                    bias=stats[0:1, 1:2],
                    scale=1.0,
                    accum_out=stats[0:1, 2:3],
                )
                nc.vector.reciprocal(out=stats[0:1, 3:4], in_=stats[0:1, 2:3])
                attn_n = soft.tile([1, SEQ], f32, tag="attn_n")
                nc.vector.tensor_scalar_mul(
                    out=attn_n[0:1, 0:se], in0=expv[0:1, 0:se], scalar1=stats[0:1, 3:4]
                )
                nc.sync.dma_start(out=attn_d[b:b + 1, 0:se], in_=attn_n[0:1, 0:se])

                # broadcast attn row across all 128 partitions: SBUF row ->
                # DRAM scratch -> stride-0 partition-replicated load
                adram = dramp.tile([1, SEQ], f32, tag="adram")
                nc.sync.dma_start(out=adram[0:1, 0:se], in_=attn_n[0:1, 0:se])
                abc = bcp.tile([P, SEQ], f32, tag="abc")
                src = adram[0:1, 0:se]
                bsrc = bass.AP(
                    tensor=src.tensor, offset=src.offset, ap=[[0, P], src.ap[-1]]
                )
                nc.sync.dma_start(out=abc[:, 0:se], in_=bsrc)

                # ctx.T[d, b] = sum_s encT[d, s] * attn[s]  (fused mul+reduce)
                for k in range(DT):
                    scr = bcp.tile([P, SEQ], f32, tag="scr")
                    nc.vector.scalar_tensor_tensor(
                        out=scr[:, 0:se],
                        in0=enc_tiles[k][:, 0:se],
                        scalar=1.0,
                        in1=abc[:, 0:se],
                        op0=Alu.mult,
                        op1=Alu.mult,
                        accum_out=ctxf_tiles[k][:, b:b + 1],
                    )

            # ---- out = tanh(cat.T.T @ W2.T) ; cat upper=ctx, lower=hidden
            # The hidden-half accumulation (k>=DT) only needs inputs that
            # are ready early, so emit it first to overlap the ctx tail.
            for k in range(DT):
                nc.vector.tensor_copy(out=catb_tiles[k][:], in_=ctxf_tiles[k][:])
            ps_o = pss.tile([BPC, OUT], f32, tag="scores", name="ps_o")
            korder = list(range(DT, CT)) + list(range(DT))
            for i, k in enumerate(korder):
                for n in range(OUT // 512):
                    nc.tensor.matmul(
                        ps_o[0:BPC, n * 512:(n + 1) * 512],
                        catb_tiles[k][:],
                        w2_tiles[k][:, n * 512:(n + 1) * 512],
                        start=(i == 0),
                        stop=(i == CT - 1),
                    )
            out2_sb = small.tile([BPC, OUT], f32, tag="out2")
            nc.scalar.activation(
                out=out2_sb[:], in_=ps_o[:], func=Act.Tanh, scale=1.0
            )
            nc.sync.dma_start(out=out2_d[:, :], in_=out2_sb[:])

    # TRN2 instructions can carry at most one sync wait (walrus rejects
    # more); split multi-wait instructions into event-semaphore pairs.
    import bass_rust as _bass_rust
    _bass_rust.generate_event_semaphores(nc)
    return nc


def _get_module(seq_effs):
    key = tuple(int(x) for x in seq_effs)
    if key not in _module_cache:
        _module_cache[key] = _build_module(key)
    return _module_cache[key]


def _plan(lens):
    """Sort batches by length desc, deal one per core per slot.
    Returns (assign[core][slot] -> batch index, seq_eff[slot])."""
    order = np.argsort(-lens, kind="stable")
    assign = [[0] * BPC for _ in range(NCORES)]
    seq_effs = []
    for g in range(BPC):
        grp = order[g * NCORES:(g + 1) * NCORES]
        gmax = int(lens[grp].max())
        seq_effs.append(min(SEQ, max(P, ((gmax + P - 1) // P) * P)))
        for c in range(NCORES):
            assign[c][g] = int(grp[c])
    return assign, tuple(seq_effs)


def prepare(hidden, encoder_outs, src_lens, W1, W2):
    """Host-side sharding/layout prep. Returns (nc, in_maps, assign)."""
    hidden = np.asarray(hidden, dtype=np.float32)
    encoder_outs = np.asarray(encoder_outs, dtype=np.float32)
    lens = np.asarray(src_lens).astype(np.int64).clip(1, SEQ)
    W1 = np.asarray(W1, dtype=np.float32)
    W2 = np.asarray(W2, dtype=np.float32)

    assign, seq_effs = _plan(lens)
    nc = _get_module(seq_effs)

    w1t = np.ascontiguousarray(W1.T)                      # [h, o]
    w2tb = np.ascontiguousarray(W2.T).astype(np.float16)  # [c, o]
    iota = np.arange(SEQ)
    addmask_full = np.where(
        iota[None, :] < lens[:, None], 0.0, NEG
    ).astype(np.float32)

    in_maps = []
    for c in range(NCORES):
        idx = assign[c]
        enc_c = encoder_outs[idx]                         # [BPC, s, d]
        enct = np.ascontiguousarray(enc_c.transpose(0, 2, 1))  # [BPC, d, s]
        hidt = np.ascontiguousarray(hidden[idx].T)        # [h, BPC]
        in_maps.append({
            "enct": enct,
            "hidt": hidt,
            "hidtb": hidt.astype(np.float16),
            "w1t": w1t,
            "w2tb": w2tb,
            "amask": np.ascontiguousarray(addmask_full[idx]).astype(
                ml_dtypes.bfloat16
            ),
        })
    return nc, in_maps, assign


def kernel(hidden, encoder_outs, src_lens, W1, W2, _trace=False):
    from concourse.bass_utils import run_bass_kernel_spmd

    nc, in_maps, assign = prepare(hidden, encoder_outs, src_lens, W1, W2)
    res = run_bass_kernel_spmd(
        nc, in_maps, core_ids=list(range(NCORES)), trace=_trace
    )

    out = np.zeros((BSZ, OUT), dtype=np.float32)
    attn = np.zeros((BSZ, SEQ), dtype=np.float32)
    for c in range(NCORES):
        r = res.results[c]
        for g in range(BPC):
            bi = assign[c][g]
            out[bi] = r["out2"][g]
            attn[bi] = r["attn"][g]

    attn_scores = np.ascontiguousarray(attn.T)            # [seq, bsz]
    if _trace:
        return (out, attn_scores), res
    return out, attn_scores
